# revision 20
# baseline (speedup 1.0000x reference)
"""CASSI forward A^T(A(x)) kernel for Trainium2, 8-core data parallel.

Reference computation (independent per batch b and row m):
    y1[l, n]  = x[b, l, m, n] * phi[l, m, n]
    y2[j]     = sum_l y1[l, j - 2l]              (j in [0, 310))
    out[l, n] = phi[l, m, n] * y2[2l + n]

On-chip layout: partitions = rows m (two 128-row tiles per batch), free
dim = (l, n).  The 28-band shift-scatter-add runs as a 5-level binary tree
of strided DVE adds.  The mask-mul writes y1 into a scratch tile laid out
with small zero gaps between paired bands so each tree level is a single
(or two) wide strided tensor_tensor op whose shifted operand reads zeros
where a block has no data — no aliased read-modify-write, no per-band op
chain.  Gaps are memset once at kernel start; level ops rewrite only data
regions.  x tiles stay dense, so loads and stores are single 3.67 MB DMAs.

Uniform-slot layout: at every level, slot width = data width + next-level
shift, so in0's right-pad zeros and in1's left-pad zeros are the SAME gap
cells and every level op is a plain 2-free-dim strided tensor_tensor:
  y1  band l (256) at 258*l                        gaps [256,258) per slot
  u   i=0..13 (258) at 262*i                       gaps [258,262)
  q   i=0..6  (262) at 278*i                       gaps [262,278), [1930,1938)
  o   i=0..2  (270) at 286*i                       gaps [270,286), [842,850)
  s   s0 (286) at 0, m1 (278) at 342               zeros [286,342)
  y2  (310) dense

Sharding: batch dim (32) split 4-per-core across 8 cores; phi replicated.
"""

import numpy as np

B, L, M, N = 32, 28, 256, 256
STRIDE = 2
NCORES = 8
BPC = B // NCORES            # batches per core
NOUT = N + STRIDE * (L - 1)  # 310
P = 128                      # partitions per row tile
Y1_W = 258 * 28              # 7224, band l at 258*l, gaps [256,258) per slot
U_W = 262 * 14               # 3668, u_i at 262*i, gaps [258,262)
Q_W = 1938                   # q_i at 278*i (uniform); gaps [262,278) per slot, [1930,1938)
O_W = 850                    # o_i at 286*i; zeros [270,286)x2, [842,850)
S_W = 620                    # s0@0 (286), zeros [286,342), m1@342 (278)
XT_BUFS = 3

_cached = {}


def _build_nc():
    import concourse.bass as bass
    import concourse.mybir as mybir
    from concourse.ap import AP
    from concourse.tile import TileContext

    f32 = mybir.dt.float32
    nc = bass.Bass()
    x = nc.dram_tensor("x", [BPC, L, M, N], f32, kind="ExternalInput")
    phi = nc.dram_tensor("phi", [L, M, N], f32, kind="ExternalInput")
    out = nc.dram_tensor("out", [BPC, L, M, N], f32, kind="ExternalOutput")

    phi_mln = phi.rearrange("l m n -> m l n")

    def sub(t, off, dims):
        """AP over tile t at element offset off with free dims [[step,count],..]."""
        full = t[:]
        return AP(full.tensor, full.offset + off,
                  [[full.ap[0][0], P]] + [list(d) for d in dims])

    with TileContext(nc) as tc:
        with (
            tc.tile_pool(name="phipool", bufs=1) as phipool,
            tc.tile_pool(name="xpool", bufs=1) as xpool,
            tc.tile_pool(name="scratch", bufs=1) as sp,
        ):
            # --- persistent tiles ------------------------------------------------
            phit = [phipool.tile([P, L * N], f32, name=f"phi{pt}", tag=f"phi{pt}")
                    for pt in range(M // P)]
            xts = [xpool.tile([P, L * N], f32, name=f"xt{i}", tag=f"xt{i}")
                   for i in range(XT_BUFS)]
            y1t = sp.tile([P, Y1_W], f32, name="y1", tag="y1")
            ut = sp.tile([P, U_W], f32, name="u", tag="u")
            qt = sp.tile([P, Q_W], f32, name="q", tag="q")
            ot = sp.tile([P, O_W], f32, name="o", tag="o")
            st = sp.tile([P, S_W], f32, name="s", tag="s")
            y2t = sp.tile([P, NOUT], f32, name="y2", tag="y2")

            # --- one-time zero-gap memsets (never written afterwards) ------------
            nc.vector.memset(sub(y1t, 256, [[258, 28], [1, 2]]), 0.0)
            nc.vector.memset(sub(ut, 258, [[262, 14], [1, 4]]), 0.0)
            nc.vector.memset(sub(qt, 262, [[278, 6], [1, 16]]), 0.0)
            nc.vector.memset(sub(qt, 1930, [[1, 8]]), 0.0)
            nc.vector.memset(sub(ot, 270, [[286, 2], [1, 16]]), 0.0)
            nc.vector.memset(sub(ot, 842, [[1, 8]]), 0.0)
            nc.vector.memset(sub(st, 286, [[1, 56]]), 0.0)

            # --- phi loads (dense (l, n) layout) ---------------------------------
            # The 3-free-dim TT encoding has room for only one sync wait, so a
            # tiny 2D copy absorbs each phi DMA wait; the mask-muls then carry
            # only their x-load wait.
            nc.sync.dma_start(
                out=phit[0][:].rearrange("p (l n) -> p l n", l=L),
                in_=phi_mln[0:P],
            )

            it = 0
            for pt in range(M // P):
                for b in range(BPC):
                    xt = xts[it % XT_BUFS]
                    it += 1
                    if it == 2:
                        # phi1 load deferred past startup so it doesn't steal
                        # HBM bandwidth from phi0/load0; ACT ring keeps the
                        # SP ring free for stores
                        nc.scalar.dma_start(
                            out=phit[1][:].rearrange("p (l n) -> p l n", l=L),
                            in_=phi_mln[P: 2 * P],
                        )
                    xt_b = sub(xt, 0, [[512, 14], [256, 2], [1, 256]])
                    nc.scalar.dma_start(
                        out=xt[:].rearrange("p (l n) -> p l n", l=L),
                        in_=x[b].rearrange("l m n -> m l n")[pt * P: (pt + 1) * P],
                    )
                    # y1 = x * phi, dense -> uniform gapped scratch
                    nc.vector.tensor_mul(
                        out=sub(y1t, 0, [[258, 28], [1, 256]]),
                        in0=sub(xt, 0, [[256, 28], [1, 256]]),
                        in1=sub(phit[pt], 0, [[256, 28], [1, 256]]),
                    )
                    # L1: 14 pair-sums -> u
                    nc.vector.tensor_add(
                        out=sub(ut, 0, [[262, 14], [1, 258]]),
                        in0=sub(y1t, 0, [[516, 14], [1, 258]]),
                        in1=sub(y1t, 256, [[516, 14], [1, 258]]),
                    )
                    # L2: 7 quad-sums -> q (single uniform op, stride 278)
                    nc.vector.tensor_add(
                        out=sub(qt, 0, [[278, 7], [1, 262]]),
                        in0=sub(ut, 0, [[524, 7], [1, 262]]),
                        in1=sub(ut, 258, [[524, 7], [1, 262]]),
                    )
                    # L3: 3 oct-sums -> o
                    nc.vector.tensor_add(
                        out=sub(ot, 0, [[286, 3], [1, 270]]),
                        in0=sub(qt, 0, [[556, 3], [1, 270]]),
                        in1=sub(qt, 270, [[556, 3], [1, 270]]),
                    )
                    # L4: s0 = o0 + shift16(o1); m1 = o2 + shift16(q6)
                    nc.vector.tensor_add(
                        out=sub(st, 0, [[1, 286]]),
                        in0=sub(ot, 0, [[1, 286]]),
                        in1=sub(ot, 270, [[1, 286]]),
                    )
                    nc.vector.tensor_add(
                        out=sub(st, 342, [[1, 278]]),
                        in0=sub(ot, 572, [[1, 278]]),
                        in1=sub(qt, 1652, [[1, 278]]),
                    )
                    # L5: y2 = s0 + shift32(m1)
                    nc.vector.tensor_add(
                        out=sub(y2t, 0, [[1, 310]]),
                        in0=sub(st, 0, [[1, 310]]),
                        in1=sub(st, 310, [[1, 310]]),
                    )
                    # out = phi * gather(y2), written into y1's data regions
                    # (y1's only writers are DVE ops, so the store below needs
                    # exactly one DVE-sem wait)
                    nc.vector.tensor_mul(
                        out=sub(xt, 0, [[256, 28], [1, 256]]),
                        in0=sub(y2t, 0, [[2, 28], [1, 256]]),
                        in1=sub(phit[pt], 0, [[256, 28], [1, 256]]),
                    )
                    o_mln = out[b].rearrange("l m n -> m l n")[pt * P: (pt + 1) * P]
                    if it < 2 * BPC:
                        # full store on the SP ring (balances the ACT ring's
                        # loads: ~33 MB per ring)
                        nc.sync.dma_start(
                            out=o_mln,
                            in_=xt[:].rearrange("p (l n) -> p l n", l=L),
                        )
                    else:
                        # last store split across both rings to halve the
                        # tail drain
                        for par, eng in ((0, nc.sync), (1, nc.scalar)):
                            eng.dma_start(
                                out=AP(o_mln.tensor, o_mln.offset + 65536 * par,
                                       [list(o_mln.ap[0]), [131072, 14], [1, 256]]),
                                in_=sub(xt, 256 * par, [[512, 14], [1, 256]]),
                            )
    _split_excess_waits(nc, mybir)
    return nc


def _split_excess_waits(nc, mybir):
    """Move all-but-one semaphore waits off capacity-limited instructions.

    The TRN2 ISA packs sync commands into each 64B instruction; multi-dim
    TT/DMA encodings have room for only one wait, and walrus codegen dies
    with "Too many sync wait commands" instead of splitting.  A standalone
    EventSemaphore on the same engine right before the op is semantically
    identical (the sequencer executes both in order)."""
    ctr = 0
    for bb in nc.m.functions[0].blocks:
        new = []
        for ins in bb.instructions:
            si = ins.sync_info
            waits = list(si.on_wait) if si is not None and si.on_wait else []
            if len(waits) > 1:
                for w in waits[:-1]:
                    ctr += 1
                    new.append(mybir.InstEventSemaphore(
                        name=f"wsplit-{ctr}",
                        engine=ins.engine,
                        sync_info=mybir.SyncInfo(on_wait=[w], on_update=[]),
                    ))
                ins.sync_info = mybir.SyncInfo(
                    on_wait=[waits[-1]],
                    on_update=list(si.on_update or []),
                )
            new.append(ins)
        bb.instructions = new


def _get_nc():
    if "nc" not in _cached:
        _cached["nc"] = _build_nc()
    return _cached["nc"]


def kernel(x: np.ndarray, phi: np.ndarray) -> np.ndarray:
    from concourse.bass_utils import run_bass_kernel_spmd

    x = np.ascontiguousarray(x, dtype=np.float32)
    phi = np.ascontiguousarray(phi, dtype=np.float32)
    assert x.shape == (B, L, M, N) and phi.shape == (L, M, N)

    nc = _get_nc()
    in_maps = [
        {"x": x[c * BPC: (c + 1) * BPC], "phi": phi} for c in range(NCORES)
    ]
    res = run_bass_kernel_spmd(nc, in_maps, core_ids=list(range(NCORES)))
    outs = [res.results[c]["out"] for c in range(NCORES)]
    return np.concatenate(outs, axis=0)


# revision 21
# speedup vs baseline: 1.0307x; 1.0307x over previous
"""CASSI forward A^T(A(x)) kernel for Trainium2, 8-core data parallel.

Reference computation (independent per batch b and row m):
    y1[l, n]  = x[b, l, m, n] * phi[l, m, n]
    y2[j]     = sum_l y1[l, j - 2l]              (j in [0, 310))
    out[l, n] = phi[l, m, n] * y2[2l + n]

On-chip layout: partitions = rows m (two 128-row tiles per batch), free
dim = (l, n).  The 28-band shift-scatter-add runs as a 5-level binary tree
of strided DVE adds.  The mask-mul writes y1 into a scratch tile laid out
with small zero gaps between paired bands so each tree level is a single
(or two) wide strided tensor_tensor op whose shifted operand reads zeros
where a block has no data — no aliased read-modify-write, no per-band op
chain.  Gaps are memset once at kernel start; level ops rewrite only data
regions.  x tiles stay dense, so loads and stores are single 3.67 MB DMAs.

Uniform-slot layout: at every level, slot width = data width + next-level
shift, so in0's right-pad zeros and in1's left-pad zeros are the SAME gap
cells and every level op is a plain 2-free-dim strided tensor_tensor:
  y1  band l (256) at 258*l                        gaps [256,258) per slot
  u   i=0..13 (258) at 262*i                       gaps [258,262)
  q   i=0..6  (262) at 278*i                       gaps [262,278), [1930,1938)
  o   i=0..2  (270) at 286*i                       gaps [270,286), [842,850)
  s   s0 (286) at 0, m1 (278) at 342               zeros [286,342)
  y2  (310) dense

Sharding: batch dim (32) split 4-per-core across 8 cores; phi replicated.
"""

import numpy as np

B, L, M, N = 32, 28, 256, 256
STRIDE = 2
NCORES = 8
BPC = B // NCORES            # batches per core
NOUT = N + STRIDE * (L - 1)  # 310
P = 128                      # partitions per row tile
Y1_W = 258 * 28              # 7224, band l at 258*l, gaps [256,258) per slot
U_W = 262 * 14               # 3668, u_i at 262*i, gaps [258,262)
Q_W = 1938                   # q_i at 278*i (uniform); gaps [262,278) per slot, [1930,1938)
O_W = 850                    # o_i at 286*i; zeros [270,286)x2, [842,850)
S_W = 620                    # s0@0 (286), zeros [286,342), m1@342 (278)
XT_BUFS = 2

_cached = {}


def _build_nc():
    import concourse.bass as bass
    import concourse.mybir as mybir
    from concourse.ap import AP
    from concourse.tile import TileContext

    f32 = mybir.dt.float32
    nc = bass.Bass()
    x = nc.dram_tensor("x", [BPC, L, M, N], f32, kind="ExternalInput")
    phi = nc.dram_tensor("phi", [L, M, N], f32, kind="ExternalInput")
    out = nc.dram_tensor("out", [BPC, L, M, N], f32, kind="ExternalOutput")

    phi_mln = phi.rearrange("l m n -> m l n")

    def sub(t, off, dims):
        """AP over tile t at element offset off with free dims [[step,count],..]."""
        full = t[:]
        return AP(full.tensor, full.offset + off,
                  [[full.ap[0][0], P]] + [list(d) for d in dims])

    with TileContext(nc) as tc:
        with (
            tc.tile_pool(name="phipool", bufs=1) as phipool,
            tc.tile_pool(name="xpool", bufs=1) as xpool,
            tc.tile_pool(name="scratch", bufs=1) as sp,
        ):
            # --- persistent tiles ------------------------------------------------
            phit = [phipool.tile([P, L * N], f32, name=f"phi{pt}", tag=f"phi{pt}")
                    for pt in range(M // P)]
            xts = [xpool.tile([P, L * N], f32, name=f"xt{i}", tag=f"xt{i}")
                   for i in range(XT_BUFS)]
            y1s = [sp.tile([P, Y1_W], f32, name=f"y1_{i}", tag=f"y1_{i}")
                   for i in range(2)]
            ut = sp.tile([P, U_W], f32, name="u", tag="u")
            qt = sp.tile([P, Q_W], f32, name="q", tag="q")
            ot = sp.tile([P, O_W], f32, name="o", tag="o")
            st = sp.tile([P, S_W], f32, name="s", tag="s")
            y2t = sp.tile([P, NOUT], f32, name="y2", tag="y2")

            # --- one-time zero-gap memsets (never written afterwards) ------------
            for y1t in y1s:
                nc.vector.memset(sub(y1t, 256, [[258, 28], [1, 2]]), 0.0)
            nc.vector.memset(sub(ut, 258, [[262, 14], [1, 4]]), 0.0)
            nc.vector.memset(sub(qt, 262, [[278, 6], [1, 16]]), 0.0)
            nc.vector.memset(sub(qt, 1930, [[1, 8]]), 0.0)
            nc.vector.memset(sub(ot, 270, [[286, 2], [1, 16]]), 0.0)
            nc.vector.memset(sub(ot, 842, [[1, 8]]), 0.0)
            nc.vector.memset(sub(st, 286, [[1, 56]]), 0.0)

            # --- phi loads (dense (l, n) layout) ---------------------------------
            # The 3-free-dim TT encoding has room for only one sync wait, so a
            # tiny 2D copy absorbs each phi DMA wait; the mask-muls then carry
            # only their x-load wait.
            nc.sync.dma_start(
                out=phit[0][:].rearrange("p (l n) -> p l n", l=L),
                in_=phi_mln[0:P],
            )

            it = 0
            for pt in range(M // P):
                for b in range(BPC):
                    xt = xts[it % XT_BUFS]
                    y1t = y1s[it % 2]
                    it += 1
                    if it == 2:
                        # phi1 load deferred past startup so it doesn't steal
                        # HBM bandwidth from phi0/load0; ACT ring keeps the
                        # SP ring free for stores
                        nc.scalar.dma_start(
                            out=phit[1][:].rearrange("p (l n) -> p l n", l=L),
                            in_=phi_mln[P: 2 * P],
                        )
                    xt_b = sub(xt, 0, [[512, 14], [256, 2], [1, 256]])
                    nc.scalar.dma_start(
                        out=xt[:].rearrange("p (l n) -> p l n", l=L),
                        in_=x[b].rearrange("l m n -> m l n")[pt * P: (pt + 1) * P],
                    )
                    # y1 = x * phi, dense -> uniform gapped scratch
                    nc.vector.tensor_mul(
                        out=sub(y1t, 0, [[258, 28], [1, 256]]),
                        in0=sub(xt, 0, [[256, 28], [1, 256]]),
                        in1=sub(phit[pt], 0, [[256, 28], [1, 256]]),
                    )
                    # L1: 14 pair-sums -> u
                    nc.vector.tensor_add(
                        out=sub(ut, 0, [[262, 14], [1, 258]]),
                        in0=sub(y1t, 0, [[516, 14], [1, 258]]),
                        in1=sub(y1t, 256, [[516, 14], [1, 258]]),
                    )
                    # L2: 7 quad-sums -> q (single uniform op, stride 278)
                    nc.vector.tensor_add(
                        out=sub(qt, 0, [[278, 7], [1, 262]]),
                        in0=sub(ut, 0, [[524, 7], [1, 262]]),
                        in1=sub(ut, 258, [[524, 7], [1, 262]]),
                    )
                    # L3: 3 oct-sums -> o
                    nc.vector.tensor_add(
                        out=sub(ot, 0, [[286, 3], [1, 270]]),
                        in0=sub(qt, 0, [[556, 3], [1, 270]]),
                        in1=sub(qt, 270, [[556, 3], [1, 270]]),
                    )
                    # L4: s0 = o0 + shift16(o1); m1 = o2 + shift16(q6)
                    nc.vector.tensor_add(
                        out=sub(st, 0, [[1, 286]]),
                        in0=sub(ot, 0, [[1, 286]]),
                        in1=sub(ot, 270, [[1, 286]]),
                    )
                    nc.vector.tensor_add(
                        out=sub(st, 342, [[1, 278]]),
                        in0=sub(ot, 572, [[1, 278]]),
                        in1=sub(qt, 1652, [[1, 278]]),
                    )
                    # L5: y2 = s0 + shift32(m1)
                    nc.vector.tensor_add(
                        out=sub(y2t, 0, [[1, 310]]),
                        in0=sub(st, 0, [[1, 310]]),
                        in1=sub(st, 310, [[1, 310]]),
                    )
                    # out = phi * gather(y2), written into y1's data regions
                    # (y1's only writers are DVE ops, so the store below needs
                    # exactly one DVE-sem wait)
                    nc.vector.tensor_mul(
                        out=sub(y1t, 0, [[258, 28], [1, 256]]),
                        in0=sub(y2t, 0, [[2, 28], [1, 256]]),
                        in1=sub(phit[pt], 0, [[256, 28], [1, 256]]),
                    )
                    o_mln = out[b].rearrange("l m n -> m l n")[pt * P: (pt + 1) * P]
                    if it < 2 * BPC:
                        # full store on the SP ring (balances the ACT ring's
                        # loads: ~33 MB per ring)
                        nc.sync.dma_start(
                            out=o_mln, in_=sub(y1t, 0, [[258, 28], [1, 256]]),
                        )
                    else:
                        # last store split across both rings to halve the
                        # tail drain
                        for par, eng in ((0, nc.sync), (1, nc.scalar)):
                            eng.dma_start(
                                out=AP(o_mln.tensor, o_mln.offset + 65536 * par,
                                       [list(o_mln.ap[0]), [131072, 14], [1, 256]]),
                                in_=sub(y1t, 258 * par, [[516, 14], [1, 256]]),
                            )
    _split_excess_waits(nc, mybir)
    return nc


def _split_excess_waits(nc, mybir):
    """Move all-but-one semaphore waits off capacity-limited instructions.

    The TRN2 ISA packs sync commands into each 64B instruction; multi-dim
    TT/DMA encodings have room for only one wait, and walrus codegen dies
    with "Too many sync wait commands" instead of splitting.  A standalone
    EventSemaphore on the same engine right before the op is semantically
    identical (the sequencer executes both in order)."""
    ctr = 0
    for bb in nc.m.functions[0].blocks:
        new = []
        for ins in bb.instructions:
            si = ins.sync_info
            waits = list(si.on_wait) if si is not None and si.on_wait else []
            if len(waits) > 1:
                for w in waits[:-1]:
                    ctr += 1
                    new.append(mybir.InstEventSemaphore(
                        name=f"wsplit-{ctr}",
                        engine=ins.engine,
                        sync_info=mybir.SyncInfo(on_wait=[w], on_update=[]),
                    ))
                ins.sync_info = mybir.SyncInfo(
                    on_wait=[waits[-1]],
                    on_update=list(si.on_update or []),
                )
            new.append(ins)
        bb.instructions = new


def _get_nc():
    if "nc" not in _cached:
        _cached["nc"] = _build_nc()
    return _cached["nc"]


def kernel(x: np.ndarray, phi: np.ndarray) -> np.ndarray:
    from concourse.bass_utils import run_bass_kernel_spmd

    x = np.ascontiguousarray(x, dtype=np.float32)
    phi = np.ascontiguousarray(phi, dtype=np.float32)
    assert x.shape == (B, L, M, N) and phi.shape == (L, M, N)

    nc = _get_nc()
    in_maps = [
        {"x": x[c * BPC: (c + 1) * BPC], "phi": phi} for c in range(NCORES)
    ]
    res = run_bass_kernel_spmd(nc, in_maps, core_ids=list(range(NCORES)))
    outs = [res.results[c]["out"] for c in range(NCORES)]
    return np.concatenate(outs, axis=0)
